# revision 27
# baseline (speedup 1.0000x reference)
"""Masked multi-head attention on 8 trn2 NeuronCores (Bass/Tile).

B=1, N=4096, C=256, H=8 (Dh=32); dense 0/1 mask shared across heads.
Sequence-parallel over query rows: core i handles query rows
[512*i, 512*(i+1)) for all heads; k/v recomputed per core from the full
x; the mask is split 8 ways (no duplication), no collectives.

v3 schedule: one step per (ktile, head-pair): 2 score matmuls into a
triple-buffered [128,1024] PSUM tile, one 1024-wide exp, 2 mask mults;
pv matmuls lag two steps (the PE is in-order) so nothing stalls behind
the exp/mask chain.  PSUM tags: s(3x2 banks) + pv(2x1) = 8 banks;
phase-1 q/k/v emissions share the s-ring (2-step WAR slack).  The
output projection accumulates per-pair into an SBUF f32 tile (DVE
adds) so the tail stays short.
"""

import sys

for _p in ("/opt/trn_rl_repo", "/root/.axon_site/_ro/trn_rl_repo"):
    if _p not in sys.path:
        sys.path.insert(0, _p)

import numpy as np
import ml_dtypes

BF16NP = ml_dtypes.bfloat16

N = 4096
C = 256
H = 8
DH = 32
NCORES = 8
NQ = N // NCORES  # 512 query rows per core
KT = N // 128  # 32 key tiles

_CACHE = {}


def build_kernel():
    import concourse.bacc as bacc
    import concourse.tile as tile
    from concourse import mybir
    import concourse.bass as bass

    F32 = mybir.dt.float32
    BF = mybir.dt.bfloat16
    EXP = mybir.ActivationFunctionType.Exp
    IDENT = mybir.ActivationFunctionType.Identity
    ADD = mybir.AluOpType.add

    nc = bacc.Bacc("TRN2", target_bir_lowering=False, debug=False, num_devices=NCORES)

    xT_d = nc.dram_tensor("xT", [C, N], BF, kind="ExternalInput")
    xqT_d = nc.dram_tensor("xqT", [C, NQ], BF, kind="ExternalInput")
    wqkv_d = nc.dram_tensor("wqkv", [C, 3 * C], BF, kind="ExternalInput")
    wproj2_d = nc.dram_tensor("wproj2", [4 * 128, C], BF, kind="ExternalInput")
    bias2_d = nc.dram_tensor("bias2", [128, 2], F32, kind="ExternalInput")
    maskT_d = nc.dram_tensor("maskT", [N, NQ], BF, kind="ExternalInput")
    out_d = nc.dram_tensor("out", [C, NQ], F32, kind="ExternalOutput")

    TAGBUFS = {"s": 3, "pv": 2}
    TAGCOLS = {"s": 1024, "pv": 512}

    with (
        tile.TileContext(nc) as tc,
        tc.tile_pool(name="consts", bufs=1) as consts,
        tc.tile_pool(name="ps", bufs=1, space="PSUM") as ps,
        tc.tile_pool(name="pp", bufs=2) as pp,
    ):
        # ---------------- input DMAs ----------------
        # sync queue: the head-phase critical path (HWDGE, low latency)
        w_sb = [
            consts.tile([128, 3 * C], BF, name=f"w_sb{c}", tag=f"w{c}")
            for c in range(2)
        ]
        for c in range(2):
            nc.sync.dma_start(out=w_sb[c], in_=wqkv_d[128 * c : 128 * (c + 1), :])
        xq_sb = [
            consts.tile([128, NQ], BF, name=f"xq_sb{c}", tag=f"xq{c}") for c in range(2)
        ]
        for c in range(2):
            nc.scalar.dma_start(out=xq_sb[c], in_=xqT_d[128 * c : 128 * (c + 1), :])
        # xT split into head/tail tiles so early consumers only depend on
        # the early DMAs (deps against DMA writes are tile-granular).
        xT_sb = [
            [
                consts.tile([128, 2048], BF, name=f"xT_sb{c}{h}", tag=f"xT{c}{h}")
                for h in range(2)
            ]
            for c in range(2)
        ]
        for c in range(2):
            nc.sync.dma_start(
                out=xT_sb[c][0][:, 0:512], in_=xT_d[128 * c : 128 * (c + 1), 0:512]
            )
        for c in range(2):
            nc.scalar.dma_start(
                out=xT_sb[c][0][:, 512:2048],
                in_=xT_d[128 * c : 128 * (c + 1), 512:2048],
            )
        # gpsimd queue: mask + bulk tail (SWDGE)
        mask_sb = consts.tile([128, KT, NQ], BF, name="mask_sb", tag="mask")
        maskT_r = maskT_d.rearrange("(m p) q -> p m q", p=128)
        nc.gpsimd.dma_start(out=mask_sb[:, 0:4, :], in_=maskT_r[:, 0:4, :])
        wp_sb = consts.tile([128, 4, C], BF, name="wp_sb", tag="wp")
        bias_sb = consts.tile([128, 2], F32, name="bias_sb", tag="bias")

        # bulk DMAs are issued INSIDE the step loop, just ahead of their
        # consumers: sem waits reference per-queue DMA watermarks at the
        # instruction's program point, so a bulk DMA issued up front makes
        # unrelated early instructions wait for it.
        def late_dmas(j, m):
            if j == 0 and m == 0:
                for c in range(2):
                    nc.scalar.dma_start(
                        out=xT_sb[c][1], in_=xT_d[128 * c : 128 * (c + 1), 2048:4096]
                    )
            if j == 0 and m % 4 == 0 and m // 4 + 1 < 8:
                ch = m // 4 + 1
                nc.gpsimd.dma_start(
                    out=mask_sb[:, 4 * ch : 4 * (ch + 1), :],
                    in_=maskT_r[:, 4 * ch : 4 * (ch + 1), :],
                )
            if j == 0 and m == 26:
                nc.gpsimd.dma_start(
                    out=wp_sb, in_=wproj2_d.rearrange("(g p) c -> p g c", p=128)
                )
            if j == 0 and m == 28:
                nc.gpsimd.dma_start(out=bias_sb, in_=bias2_d[:])

        # ---------------- persistent SBUF ----------------
        qT_sb = [
            consts.tile([128, NQ], BF, name=f"qT_sb{g}", tag=f"qT{g}") for g in range(2)
        ]
        kT_sb = [
            consts.tile([128, N], BF, name=f"kT_sb{g}", tag=f"kT{g}") for g in range(2)
        ]
        # v tiles with a fused ones column: per ktile, 34-wide blocks
        # [v_h (32) | 1 | pad] so lhsT [128, 33] per head fuses the softmax
        # denominator into the pv matmul as output row 32.
        v_all = consts.tile([128, KT, 34 * H], BF, name="v_all", tag="vall")
        rec_sb = consts.tile([128, 4 * NQ], F32, name="rec_sb", tag="rec")
        o_cat = [
            consts.tile([128, NQ], BF, name=f"o_cat{j}", tag=f"oc{j}") for j in range(4)
        ]
        ones_sb = consts.tile([128, 64], F32, name="ones_sb", tag="ones")
        bc_sb = consts.tile([128, NQ], F32, name="bc_sb", tag="bcs")
        out_sb = consts.tile([128, 1024], F32, name="out_sb", tag="osb")

        nc.vector.memset(ones_sb, 1.0)
        v_r = v_all.rearrange("p m (h w) -> p m h w", h=H)
        for h in range(H):
            nc.vector.memset(v_r[:, :, h, 32:34], 1.0)

        # ---------------- phase-1 emissions ----------------
        def emit_q(g, tag):
            q_ps = ps.tile(
                [128, TAGCOLS[tag]], F32, name="q_ps", tag=tag, bufs=TAGBUFS[tag]
            )
            for c in range(2):
                nc.tensor.matmul(
                    out=q_ps[:, 0:NQ],
                    lhsT=w_sb[c][:, 128 * g : 128 * (g + 1)],
                    rhs=xq_sb[c],
                    start=(c == 0),
                    stop=(c == 1),
                )
            nc.vector.tensor_copy(out=qT_sb[g], in_=q_ps[:, 0:NQ])

        def emit_kT(g, n, tag):
            k_ps = ps.tile(
                [128, TAGCOLS[tag]], F32, name="k_ps", tag=tag, bufs=TAGBUFS[tag]
            )
            for c in range(2):
                nc.tensor.matmul(
                    out=k_ps[:, 0:512],
                    lhsT=w_sb[c][:, 256 + 128 * g : 256 + 128 * (g + 1)],
                    rhs=xT_sb[c][n // 4][:, 512 * (n % 4) : 512 * (n % 4 + 1)],
                    start=(c == 0),
                    stop=(c == 1),
                )
            nc.vector.tensor_copy(
                out=kT_sb[g][:, 512 * n : 512 * (n + 1)], in_=k_ps[:, 0:512]
            )

        def emit_v2(m, tag):
            # two ktiles (m, m+1) per emission slot: 4 matmuls, one copy
            v_ps = ps.tile(
                [128, TAGCOLS[tag]], F32, name="v_ps", tag=tag, bufs=TAGBUFS[tag]
            )
            for k2 in range(2):
                mk = m + k2
                for c in range(2):
                    nc.tensor.matmul(
                        out=v_ps[:, C * k2 : C * (k2 + 1)],
                        lhsT=xT_sb[c][mk // 16][
                            :, 128 * (mk % 16) : 128 * (mk % 16 + 1)
                        ],
                        rhs=w_sb[c][:, 512:768],
                        start=(c == 0),
                        stop=(c == 1),
                    )
            nc.vector.tensor_copy(
                out=v_r[:, m : m + 2, :, 0:32],
                in_=v_ps[:, 0 : 2 * C].rearrange("p (m2 h w) -> p m2 h w", m2=2, h=H),
            )

        # head: the minimum to start attention: q(g0), v(0..3), kT(0,0)
        head_ems = [
            (lambda n: lambda t: emit_kT(0, n, t))(0),
            lambda t: emit_q(0, t),
            (lambda m: lambda t: emit_v2(m, t))(0),
            (lambda m: lambda t: emit_v2(m, t))(2),
        ]
        ring = ["s", "pv", "s", "pv"]
        for i, em in enumerate(head_ems):
            em(ring[i])

        # per-pair emission queues: (closure, needed_by_step, strict) —
        # popped at most 2 per step onto the shared "s" ring.  v-emissions
        # are consumed by pv (which lags 2 steps), so their need is lax.
        emq = {j: [] for j in range(4)}
        for mm in range(4, KT, 2):
            emq[0].append(((lambda m: lambda t: emit_v2(m, t))(mm), mm, False))
        for n in range(1, 8):
            emq[0].append(((lambda n_: lambda t: emit_kT(0, n_, t))(n), 4 * n, True))
        emq[0].sort(key=lambda e: e[1])
        emq[1] = [
            ((lambda n_: lambda t: emit_kT(1, n_, t))(n), 99, True) for n in range(8)
        ]
        emq[1].append((lambda t: emit_q(1, t), 99, True))

        # ---------------- phase 2: attention ----------------
        pv_tiles = {}

        def get_pv(j):
            if j not in pv_tiles:
                t = ps.tile([128, 512], F32, name="pv_t", tag="pv", bufs=2)
                pv_tiles[j] = t
                if j < 2:
                    # 1.0 keeps reciprocal_approx_fast well-defined on rows
                    # the pv matmuls never write; later pairs reuse the slot,
                    # whose unwritten rows still hold this memset.
                    nc.vector.memset(t, 1.0)
            return pv_tiles[j]

        def issue_pv(j, m, p_t):
            hA, hB = 2 * j, 2 * j + 1
            pv_t = get_pv(j)
            for h, base in ((hA, 0), (hB, 1)):
                rows = (0, 33) if base == 0 else (64, 97)
                tp = (0, 0) if base == 0 else (0, 64)
                nc.tensor.matmul(
                    out=pv_t[rows[0] : rows[1], 0:NQ],
                    lhsT=v_all[:, m, 34 * h : 34 * h + 33],
                    rhs=p_t[:, 512 * base : 512 * base + NQ],
                    start=(m == 0),
                    stop=(m == KT - 1),
                    tile_position=tp,
                    skip_group_check=True,
                )

        def epi_rec(j):
            pv_t = pv_tiles[j]
            nc.vector.reciprocal_approx_fast(
                out=rec_sb[:, NQ * j : NQ * j + NQ], in_=pv_t[:, 0:NQ]
            )

        def epi_fin(j):
            # per-head softmax normalization: broadcast the reciprocal of
            # the fused denominator rows (32 / 96) across the head's 64
            # partitions with a ones-column matmul (contraction 1), then a
            # columnwise multiply straight out of PSUM.
            pv_t = pv_tiles.pop(j)
            bc_ps = ps.tile([128, 1024], F32, name="bc_ps", tag="s", bufs=3)
            for prow, orow in ((32, 0), (96, 64)):
                nc.tensor.matmul(
                    out=bc_ps[orow : orow + 64, 0:NQ],
                    lhsT=ones_sb[prow : prow + 1, 0:64],
                    rhs=rec_sb[prow : prow + 1, NQ * j : NQ * j + NQ],
                    start=True,
                    stop=True,
                    tile_position=(prow, orow),
                    skip_group_check=True,
                )
            nc.vector.tensor_copy(out=bc_sb, in_=bc_ps[:, 0:NQ])
            nc.vector.tensor_mul(
                out=o_cat[j],
                in0=pv_t[:, 0:NQ],
                in1=bc_sb,
            )

        # step list: one step per (pair, ktile)
        all_steps = []
        for j in range(4):
            for m in range(KT):
                all_steps.append((j, m))

        # pv matmuls lag TWO steps behind the step whose p they read: the
        # PE is in-order, so a shorter lag stalls it behind exp+mask.
        pend = []
        fin_pend = []
        for j, m in all_steps:
            late_dmas(j, m)
            hA, hB = 2 * j, 2 * j + 1
            gA = hA // 4
            pA, pB = 32 * (hA % 4), 32 * (hB % 4)
            s_t = ps.tile([128, 1024], F32, name="s_t", tag="s", bufs=3)
            p_t = pp.tile([128, 1024], BF, name="p_t", tag="p", bufs=4)
            # scores: out (kpos, qrow) per head; hA -> cols 0:512, hB -> 512:1024
            for hi, p_off in ((0, pA), (1, pB)):
                nc.tensor.matmul(
                    out=s_t[:, 512 * hi : 512 * hi + NQ],
                    lhsT=kT_sb[gA][p_off : p_off + 32, 128 * m : 128 * (m + 1)],
                    rhs=qT_sb[gA][p_off : p_off + 32, :],
                    start=True,
                    stop=True,
                    tile_position=(p_off, 0),
                )
            nc.scalar.activation(out=p_t, in_=s_t, func=EXP)
            nc.vector.tensor_mul(
                out=p_t[:, 0:512], in0=p_t[:, 0:512], in1=mask_sb[:, m, :]
            )
            nc.vector.tensor_mul(
                out=p_t[:, 512:1024], in0=p_t[:, 512:1024], in1=mask_sb[:, m, :]
            )
            if len(pend) == 2:
                pj, pm, pp_t = pend.pop(0)
                issue_pv(pj, pm, pp_t)
                if pm == KT - 1:
                    epi_rec(pj)
                    fin_pend.append((pj, 2))
            for i in range(len(fin_pend) - 1, -1, -1):
                fj, cnt = fin_pend[i]
                if cnt <= 0:
                    epi_fin(fj)
                    fin_pend.pop(i)
                else:
                    fin_pend[i] = (fj, cnt - 1)
            for _ in range(2):
                if emq[j]:
                    em, need, strict = emq[j].pop(0)
                    assert (need > m) if strict else (need + 1 > m), (
                        f"emission needed at step {need} popped too late "
                        f"(step {m}) for pair {j}"
                    )
                    em("s")
            pend.append((j, m, p_t))

        for pj, pm, pp_t in pend:
            issue_pv(pj, pm, pp_t)
            if pm == KT - 1:
                epi_rec(pj)
                fin_pend.append((pj, 0))
        for fj, _ in fin_pend:
            epi_fin(fj)

        # final projection: accumulate all four pairs in PSUM at the end
        # (o_cat[0..2] are long ready; only o_cat[3] gates this).
        f_ps = ps.tile([128, 1024], F32, name="f_ps", tag="s", bufs=3)
        for t in range(2):
            for j in range(4):
                nc.tensor.matmul(
                    out=f_ps[:, 512 * t : 512 * (t + 1)],
                    lhsT=wp_sb[:, j, 128 * t : 128 * (t + 1)],
                    rhs=o_cat[j],
                    start=(j == 0),
                    stop=(j == 3),
                )
        for t in range(2):
            nc.scalar.activation(
                out=out_sb[:, 512 * t : 512 * (t + 1)],
                in_=f_ps[:, 512 * t : 512 * (t + 1)],
                func=IDENT,
                bias=bias_sb[:, t : t + 1],
                scale=1.0,
            )
        nc.sync.dma_start(out=out_d[0:128, :], in_=out_sb[:, 0:512])
        nc.scalar.dma_start(out=out_d[128:256, :], in_=out_sb[:, 512:1024])

    nc.compile()
    return nc


def _get_nc():
    if "nc" not in _CACHE:
        _CACHE["nc"] = build_kernel()
    return _CACHE["nc"]


def _prep_inputs(x, adj, w_qkv, w_proj, b_proj):
    x = np.asarray(x, dtype=np.float32).reshape(N, C)
    adj = np.asarray(adj).reshape(N, N)
    w_qkv = np.asarray(w_qkv, dtype=np.float32)
    w_proj = np.asarray(w_proj, dtype=np.float32)
    b_proj = np.asarray(b_proj, dtype=np.float32)

    scale = float(DH) ** -0.5
    wqkvT = w_qkv.T.copy()
    wqkvT[:, 0:C] *= scale  # fold attention scale into q projection
    wqkvT = np.ascontiguousarray(wqkvT, dtype=BF16NP)
    wprojT = w_proj.T.astype(np.float32)  # [C (contraction), C (out)]
    # zero-padded reorder: block j rows 0:32 = head 2j, rows 64:96 =
    # head 2j+1, rest zero (matches pv bank partition layout)
    wproj2 = np.zeros((4 * 128, C), dtype=np.float32)
    for j in range(4):
        wproj2[128 * j + 0 : 128 * j + 32] = wprojT[64 * j : 64 * j + 32]
        wproj2[128 * j + 64 : 128 * j + 96] = wprojT[64 * j + 32 : 64 * j + 64]
    wproj2 = np.ascontiguousarray(wproj2, dtype=BF16NP)
    bias2 = np.ascontiguousarray(b_proj.reshape(2, 128).T, dtype=np.float32)
    xT = np.ascontiguousarray(x.T, dtype=BF16NP)
    adjT = (adj > 0).astype(BF16NP).T  # [kpos, qrow] 0/1

    in_maps = []
    for i in range(NCORES):
        sl = slice(NQ * i, NQ * (i + 1))
        in_maps.append(
            {
                "xT": xT,
                "xqT": np.ascontiguousarray(xT[:, sl]),
                "wqkv": wqkvT,
                "wproj2": wproj2,
                "bias2": bias2,
                "maskT": np.ascontiguousarray(adjT[:, sl]),
            }
        )
    return in_maps


def run_on_hw(inputs, trace=False):
    from concourse.bass_utils import run_bass_kernel_spmd

    if trace:
        import axon_profile_shim  # noqa: F401

    nc = _get_nc()
    in_maps = _prep_inputs(**inputs)
    res = run_bass_kernel_spmd(
        nc, in_maps, core_ids=list(range(NCORES)), trace=trace
    )
    out = np.empty((1, N, C), dtype=np.float32)
    for i in range(NCORES):
        out[0, NQ * i : NQ * (i + 1), :] = res.results[i]["out"].T
    return out, res


def kernel(x, adj, w_qkv, w_proj, b_proj):
    out, _ = run_on_hw(
        {"x": x, "adj": adj, "w_qkv": w_qkv, "w_proj": w_proj, "b_proj": b_proj}
    )
    return out


# revision 28
# speedup vs baseline: 1.1751x; 1.1751x over previous
"""Masked multi-head attention on 8 trn2 NeuronCores (Bass/Tile).

B=1, N=4096, C=256, H=8 (Dh=32); dense 0/1 mask shared across heads.
Sequence-parallel over query rows: core i handles query rows
[512*i, 512*(i+1)) for all heads; k/v recomputed per core from the full
x; the mask is split 8 ways (no duplication), no collectives.

v3 schedule: one step per (ktile, head-pair): 2 score matmuls into a
triple-buffered [128,1024] PSUM tile, one 1024-wide exp, 2 mask mults;
pv matmuls lag two steps (the PE is in-order) so nothing stalls behind
the exp/mask chain.  PSUM tags: s(3x2 banks) + pv(2x1) = 8 banks;
phase-1 q/k/v emissions share the s-ring (2-step WAR slack).  The
output projection accumulates per-pair into an SBUF f32 tile (DVE
adds) so the tail stays short.
"""

import sys

for _p in ("/opt/trn_rl_repo", "/root/.axon_site/_ro/trn_rl_repo"):
    if _p not in sys.path:
        sys.path.insert(0, _p)

import numpy as np
import ml_dtypes

BF16NP = ml_dtypes.bfloat16

N = 4096
C = 256
H = 8
DH = 32
NCORES = 8
NQ = N // NCORES  # 512 query rows per core
KT = N // 128  # 32 key tiles

_CACHE = {}


def build_kernel():
    import concourse.bacc as bacc
    import concourse.tile as tile
    from concourse import mybir
    import concourse.bass as bass

    F32 = mybir.dt.float32
    BF = mybir.dt.bfloat16
    EXP = mybir.ActivationFunctionType.Exp
    IDENT = mybir.ActivationFunctionType.Identity
    ADD = mybir.AluOpType.add

    nc = bacc.Bacc("TRN2", target_bir_lowering=False, debug=False, num_devices=NCORES)

    xT_d = nc.dram_tensor("xT", [C, N], BF, kind="ExternalInput")
    xqT_d = nc.dram_tensor("xqT", [C, NQ], BF, kind="ExternalInput")
    wqkv_d = nc.dram_tensor("wqkv", [C, 3 * C], BF, kind="ExternalInput")
    wproj2_d = nc.dram_tensor("wproj2", [4 * 128, C], BF, kind="ExternalInput")
    bias2_d = nc.dram_tensor("bias2", [128, 2], F32, kind="ExternalInput")
    maskT_d = nc.dram_tensor("maskT", [N, NQ], BF, kind="ExternalInput")
    out_d = nc.dram_tensor("out", [C, NQ], F32, kind="ExternalOutput")

    TAGBUFS = {"s": 3, "pv": 2}
    TAGCOLS = {"s": 1024, "pv": 512}

    with (
        tile.TileContext(nc) as tc,
        tc.tile_pool(name="consts", bufs=1) as consts,
        tc.tile_pool(name="ps", bufs=1, space="PSUM") as ps,
        tc.tile_pool(name="pp", bufs=2) as pp,
    ):
        # ---------------- input DMAs ----------------
        # sync queue: the head-phase critical path (HWDGE, low latency)
        w_sb = [
            consts.tile([128, 3 * C], BF, name=f"w_sb{c}", tag=f"w{c}")
            for c in range(2)
        ]
        for c in range(2):
            nc.sync.dma_start(out=w_sb[c], in_=wqkv_d[128 * c : 128 * (c + 1), :])
        xq_sb = [
            consts.tile([128, NQ], BF, name=f"xq_sb{c}", tag=f"xq{c}") for c in range(2)
        ]
        for c in range(2):
            nc.scalar.dma_start(out=xq_sb[c], in_=xqT_d[128 * c : 128 * (c + 1), :])
        # xT split into head/tail tiles so early consumers only depend on
        # the early DMAs (deps against DMA writes are tile-granular).
        xT_sb = [
            [
                consts.tile([128, 2048], BF, name=f"xT_sb{c}{h}", tag=f"xT{c}{h}")
                for h in range(2)
            ]
            for c in range(2)
        ]
        for c in range(2):
            nc.sync.dma_start(
                out=xT_sb[c][0][:, 0:512], in_=xT_d[128 * c : 128 * (c + 1), 0:512]
            )
        for c in range(2):
            nc.scalar.dma_start(
                out=xT_sb[c][0][:, 512:2048],
                in_=xT_d[128 * c : 128 * (c + 1), 512:2048],
            )
        # gpsimd queue: mask + bulk tail (SWDGE)
        mask_sb = consts.tile([128, KT, NQ], BF, name="mask_sb", tag="mask")
        maskT_r = maskT_d.rearrange("(m p) q -> p m q", p=128)
        wp_sb = consts.tile([128, 4, C], BF, name="wp_sb", tag="wp")
        bias_sb = consts.tile([128, 2], F32, name="bias_sb", tag="bias")

        # bulk DMAs are issued INSIDE the step loop, just ahead of their
        # consumers: sem waits reference per-queue DMA watermarks at the
        # instruction's program point, so a bulk DMA issued up front makes
        # unrelated early instructions wait for it.
        def late_dmas(j, m):
            if j == 0 and m == 0:
                for c in range(2):
                    nc.scalar.dma_start(
                        out=xT_sb[c][1], in_=xT_d[128 * c : 128 * (c + 1), 2048:4096]
                    )
            if j == 0 and m in (0, 6, 14):
                ch = {0: 1, 6: 2, 14: 3}[m]
                nc.gpsimd.dma_start(
                    out=mask_sb[:, 8 * ch : 8 * (ch + 1), :],
                    in_=maskT_r[:, 8 * ch : 8 * (ch + 1), :],
                )
            if j == 0 and m == 26:
                nc.gpsimd.dma_start(
                    out=wp_sb, in_=wproj2_d.rearrange("(g p) c -> p g c", p=128)
                )
            if j == 0 and m == 28:
                nc.gpsimd.dma_start(out=bias_sb, in_=bias2_d[:])

        # ---------------- persistent SBUF ----------------
        qT_sb = [
            consts.tile([128, NQ], BF, name=f"qT_sb{g}", tag=f"qT{g}") for g in range(2)
        ]
        kT_sb = [
            consts.tile([128, N], BF, name=f"kT_sb{g}", tag=f"kT{g}") for g in range(2)
        ]
        # v tiles with a fused ones column: per ktile, 34-wide blocks
        # [v_h (32) | 1 | pad] so lhsT [128, 33] per head fuses the softmax
        # denominator into the pv matmul as output row 32.
        v_all = consts.tile([128, KT, 34 * H], BF, name="v_all", tag="vall")
        rec_sb = consts.tile([128, 4 * NQ], F32, name="rec_sb", tag="rec")
        o_cat = [
            consts.tile([128, NQ], BF, name=f"o_cat{j}", tag=f"oc{j}") for j in range(4)
        ]
        ones_sb = consts.tile([128, 64], F32, name="ones_sb", tag="ones")
        bc_sb = consts.tile([128, NQ], F32, name="bc_sb", tag="bcs")
        out_sb = consts.tile([128, 1024], F32, name="out_sb", tag="osb")

        nc.vector.memset(ones_sb, 1.0)
        v_r = v_all.rearrange("p m (h w) -> p m h w", h=H)
        for h in range(H):
            nc.vector.memset(v_r[:, :, h, 32:34], 1.0)

        # ---------------- phase-1 emissions ----------------
        def emit_q(g, tag):
            q_ps = ps.tile(
                [128, TAGCOLS[tag]], F32, name="q_ps", tag=tag, bufs=TAGBUFS[tag]
            )
            for c in range(2):
                nc.tensor.matmul(
                    out=q_ps[:, 0:NQ],
                    lhsT=w_sb[c][:, 128 * g : 128 * (g + 1)],
                    rhs=xq_sb[c],
                    start=(c == 0),
                    stop=(c == 1),
                )
            nc.vector.tensor_copy(out=qT_sb[g], in_=q_ps[:, 0:NQ])

        def emit_kT(g, n, tag):
            k_ps = ps.tile(
                [128, TAGCOLS[tag]], F32, name="k_ps", tag=tag, bufs=TAGBUFS[tag]
            )
            for c in range(2):
                nc.tensor.matmul(
                    out=k_ps[:, 0:512],
                    lhsT=w_sb[c][:, 256 + 128 * g : 256 + 128 * (g + 1)],
                    rhs=xT_sb[c][n // 4][:, 512 * (n % 4) : 512 * (n % 4 + 1)],
                    start=(c == 0),
                    stop=(c == 1),
                )
            nc.vector.tensor_copy(
                out=kT_sb[g][:, 512 * n : 512 * (n + 1)], in_=k_ps[:, 0:512]
            )

        def emit_v2(m, tag):
            # two ktiles (m, m+1) per emission slot: 4 matmuls, one copy
            v_ps = ps.tile(
                [128, TAGCOLS[tag]], F32, name="v_ps", tag=tag, bufs=TAGBUFS[tag]
            )
            for k2 in range(2):
                mk = m + k2
                for c in range(2):
                    nc.tensor.matmul(
                        out=v_ps[:, C * k2 : C * (k2 + 1)],
                        lhsT=xT_sb[c][mk // 16][
                            :, 128 * (mk % 16) : 128 * (mk % 16 + 1)
                        ],
                        rhs=w_sb[c][:, 512:768],
                        start=(c == 0),
                        stop=(c == 1),
                    )
            nc.vector.tensor_copy(
                out=v_r[:, m : m + 2, :, 0:32],
                in_=v_ps[:, 0 : 2 * C].rearrange("p (m2 h w) -> p m2 h w", m2=2, h=H),
            )

        # head: the minimum to start attention: q(g0), v(0..3), kT(0,0)
        emit_kT(0, 0, "s")
        emit_q(0, "pv")
        nc.gpsimd.dma_start(out=mask_sb[:, 0:8, :], in_=maskT_r[:, 0:8, :])
        emit_v2(0, "s")
        emit_v2(2, "pv")

        # per-pair emission queues: (closure, needed_by_step, strict) —
        # popped at most 2 per step onto the shared "s" ring.  v-emissions
        # are consumed by pv (which lags 2 steps), so their need is lax.
        emq = {j: [] for j in range(4)}
        for mm in range(4, KT, 2):
            emq[0].append(((lambda m: lambda t: emit_v2(m, t))(mm), mm, False))
        for n in range(1, 8):
            emq[0].append(((lambda n_: lambda t: emit_kT(0, n_, t))(n), 4 * n, True))
        emq[0].sort(key=lambda e: e[1])
        emq[1] = [
            ((lambda n_: lambda t: emit_kT(1, n_, t))(n), 99, True) for n in range(8)
        ]
        emq[1].append((lambda t: emit_q(1, t), 99, True))

        # ---------------- phase 2: attention ----------------
        pv_tiles = {}

        def get_pv(j):
            if j not in pv_tiles:
                t = ps.tile([128, 512], F32, name="pv_t", tag="pv", bufs=2)
                pv_tiles[j] = t
                if j < 2:
                    # 1.0 keeps reciprocal_approx_fast well-defined on rows
                    # the pv matmuls never write; later pairs reuse the slot,
                    # whose unwritten rows still hold this memset.
                    nc.vector.memset(t, 1.0)
            return pv_tiles[j]

        def issue_pv(j, m, p_t):
            hA, hB = 2 * j, 2 * j + 1
            pv_t = get_pv(j)
            for h, base in ((hA, 0), (hB, 1)):
                rows = (0, 33) if base == 0 else (64, 97)
                tp = (0, 0) if base == 0 else (0, 64)
                nc.tensor.matmul(
                    out=pv_t[rows[0] : rows[1], 0:NQ],
                    lhsT=v_all[:, m, 34 * h : 34 * h + 33],
                    rhs=p_t[:, 512 * base : 512 * base + NQ],
                    start=(m == 0),
                    stop=(m == KT - 1),
                    tile_position=tp,
                    skip_group_check=True,
                )

        def epi_rec(j):
            pv_t = pv_tiles[j]
            nc.vector.reciprocal_approx_fast(
                out=rec_sb[:, NQ * j : NQ * j + NQ], in_=pv_t[:, 0:NQ]
            )

        def epi_fin(j):
            # per-head softmax normalization: broadcast the reciprocal of
            # the fused denominator rows (32 / 96) across the head's 64
            # partitions with a ones-column matmul (contraction 1), then a
            # columnwise multiply straight out of PSUM.
            pv_t = pv_tiles.pop(j)
            bc_ps = ps.tile([128, 1024], F32, name="bc_ps", tag="s", bufs=3)
            for prow, orow in ((32, 0), (96, 64)):
                nc.tensor.matmul(
                    out=bc_ps[orow : orow + 64, 0:NQ],
                    lhsT=ones_sb[prow : prow + 1, 0:64],
                    rhs=rec_sb[prow : prow + 1, NQ * j : NQ * j + NQ],
                    start=True,
                    stop=True,
                    tile_position=(prow, orow),
                    skip_group_check=True,
                )
            nc.vector.tensor_copy(out=bc_sb, in_=bc_ps[:, 0:NQ])
            nc.vector.tensor_mul(
                out=o_cat[j],
                in0=pv_t[:, 0:NQ],
                in1=bc_sb,
            )

        # step list: one step per (pair, ktile)
        all_steps = []
        for j in range(4):
            for m in range(KT):
                all_steps.append((j, m))

        # pv matmuls lag TWO steps behind the step whose p they read: the
        # PE is in-order, so a shorter lag stalls it behind exp+mask.
        pend = []
        fin_pend = []
        for j, m in all_steps:
            late_dmas(j, m)
            hA, hB = 2 * j, 2 * j + 1
            gA = hA // 4
            pA, pB = 32 * (hA % 4), 32 * (hB % 4)
            s_t = ps.tile([128, 1024], F32, name="s_t", tag="s", bufs=3)
            p_t = pp.tile([128, 1024], BF, name="p_t", tag="p", bufs=4)
            # scores: out (kpos, qrow) per head; hA -> cols 0:512, hB -> 512:1024
            for hi, p_off in ((0, pA), (1, pB)):
                nc.tensor.matmul(
                    out=s_t[:, 512 * hi : 512 * hi + NQ],
                    lhsT=kT_sb[gA][p_off : p_off + 32, 128 * m : 128 * (m + 1)],
                    rhs=qT_sb[gA][p_off : p_off + 32, :],
                    start=True,
                    stop=True,
                    tile_position=(p_off, 0),
                )
            nc.scalar.activation(out=p_t, in_=s_t, func=EXP)
            nc.vector.tensor_mul(
                out=p_t[:, 0:512], in0=p_t[:, 0:512], in1=mask_sb[:, m, :]
            )
            nc.vector.tensor_mul(
                out=p_t[:, 512:1024], in0=p_t[:, 512:1024], in1=mask_sb[:, m, :]
            )
            if len(pend) == 2:
                pj, pm, pp_t = pend.pop(0)
                issue_pv(pj, pm, pp_t)
                if pm == KT - 1:
                    epi_rec(pj)
                    fin_pend.append((pj, 2))
            for i in range(len(fin_pend) - 1, -1, -1):
                fj, cnt = fin_pend[i]
                if cnt <= 0:
                    epi_fin(fj)
                    fin_pend.pop(i)
                else:
                    fin_pend[i] = (fj, cnt - 1)
            for _ in range(2):
                if emq[j]:
                    em, need, strict = emq[j].pop(0)
                    assert (need > m) if strict else (need + 1 > m), (
                        f"emission needed at step {need} popped too late "
                        f"(step {m}) for pair {j}"
                    )
                    em("s")
            pend.append((j, m, p_t))

        for pj, pm, pp_t in pend:
            issue_pv(pj, pm, pp_t)
            if pm == KT - 1:
                epi_rec(pj)
                fin_pend.append((pj, 0))
        for fj, _ in fin_pend:
            epi_fin(fj)

        # final projection: accumulate all four pairs in PSUM at the end
        # (o_cat[0..2] are long ready; only o_cat[3] gates this).
        f_ps = ps.tile([128, 1024], F32, name="f_ps", tag="s", bufs=3)
        for t in range(2):
            for j in range(4):
                nc.tensor.matmul(
                    out=f_ps[:, 512 * t : 512 * (t + 1)],
                    lhsT=wp_sb[:, j, 128 * t : 128 * (t + 1)],
                    rhs=o_cat[j],
                    start=(j == 0),
                    stop=(j == 3),
                )
        for t in range(2):
            nc.scalar.activation(
                out=out_sb[:, 512 * t : 512 * (t + 1)],
                in_=f_ps[:, 512 * t : 512 * (t + 1)],
                func=IDENT,
                bias=bias_sb[:, t : t + 1],
                scale=1.0,
            )
        nc.sync.dma_start(out=out_d[0:128, :], in_=out_sb[:, 0:512])
        nc.scalar.dma_start(out=out_d[128:256, :], in_=out_sb[:, 512:1024])

    nc.compile()
    return nc


def _get_nc():
    if "nc" not in _CACHE:
        _CACHE["nc"] = build_kernel()
    return _CACHE["nc"]


def _prep_inputs(x, adj, w_qkv, w_proj, b_proj):
    x = np.asarray(x, dtype=np.float32).reshape(N, C)
    adj = np.asarray(adj).reshape(N, N)
    w_qkv = np.asarray(w_qkv, dtype=np.float32)
    w_proj = np.asarray(w_proj, dtype=np.float32)
    b_proj = np.asarray(b_proj, dtype=np.float32)

    scale = float(DH) ** -0.5
    wqkvT = w_qkv.T.copy()
    wqkvT[:, 0:C] *= scale  # fold attention scale into q projection
    wqkvT = np.ascontiguousarray(wqkvT, dtype=BF16NP)
    wprojT = w_proj.T.astype(np.float32)  # [C (contraction), C (out)]
    # zero-padded reorder: block j rows 0:32 = head 2j, rows 64:96 =
    # head 2j+1, rest zero (matches pv bank partition layout)
    wproj2 = np.zeros((4 * 128, C), dtype=np.float32)
    for j in range(4):
        wproj2[128 * j + 0 : 128 * j + 32] = wprojT[64 * j : 64 * j + 32]
        wproj2[128 * j + 64 : 128 * j + 96] = wprojT[64 * j + 32 : 64 * j + 64]
    wproj2 = np.ascontiguousarray(wproj2, dtype=BF16NP)
    bias2 = np.ascontiguousarray(b_proj.reshape(2, 128).T, dtype=np.float32)
    xT = np.ascontiguousarray(x.T, dtype=BF16NP)
    adjT = (adj > 0).astype(BF16NP).T  # [kpos, qrow] 0/1

    in_maps = []
    for i in range(NCORES):
        sl = slice(NQ * i, NQ * (i + 1))
        in_maps.append(
            {
                "xT": xT,
                "xqT": np.ascontiguousarray(xT[:, sl]),
                "wqkv": wqkvT,
                "wproj2": wproj2,
                "bias2": bias2,
                "maskT": np.ascontiguousarray(adjT[:, sl]),
            }
        )
    return in_maps


def run_on_hw(inputs, trace=False):
    from concourse.bass_utils import run_bass_kernel_spmd

    if trace:
        import axon_profile_shim  # noqa: F401

    nc = _get_nc()
    in_maps = _prep_inputs(**inputs)
    res = run_bass_kernel_spmd(
        nc, in_maps, core_ids=list(range(NCORES)), trace=trace
    )
    out = np.empty((1, N, C), dtype=np.float32)
    for i in range(NCORES):
        out[0, NQ * i : NQ * (i + 1), :] = res.results[i]["out"].T
    return out, res


def kernel(x, adj, w_qkv, w_proj, b_proj):
    out, _ = run_on_hw(
        {"x": x, "adj": adj, "w_qkv": w_qkv, "w_proj": w_proj, "b_proj": b_proj}
    )
    return out
